# revision 56
# baseline (speedup 1.0000x reference)
"""Trainium2 Bass kernel for a BinaryNet conv block.

Pipeline (per core, data-parallel over batch):
  sign(x) -> conv3x3(sign(w1)) -> BN1 -> sign -> conv3x3(sign(w2))
          -> maxpool2x2 -> BN2

Implementation notes:
  - Activations are +-0.5, weights +-1.0 in fp8e4 (exactly representable);
    convs run as per-row 56-col matmuls with DoubleRow perf mode (K=256
    contraction per instruction), accumulating exactly into fp32 PSUM.
    Per-row emission skips the 2 pad columns per row; rows sit at a
    64-col PSUM pitch so no matmul output straddles a 2KB bank and each
    row's accumulation group opens/closes before the next row reuses the
    bank (row-outer, tap-inner).
  - BN1+sign is fused into one ScalarE Sign activation against a
    host-precomputed per-channel threshold. Conv outputs are exact
    integers, so an integer cutoff k_c reproduces the reference's fp32
    sign decisions bit-exactly.
  - The host marshals x to channel-major [C, H*W] per image and reads y
    back channel-major [2, 128, PO]; the device never transposes. The PE
    therefore runs conv matmuls only, fed by a fused sign+scatter
    tensor_scalar (fp32 -> fp8 +-0.5, j0 on DVE, j1 on GPSIMD) writing
    straight into the zero-bordered padded layout.
  - Spatial layout is channel-major [ci, y*(W+2)+x] with a zero border so
    the 9 taps are just constant AP offsets.
  - Dependency tracking treats a multi-plane read as one interval hull,
    so a j0-plane prep write emitted before a conv instruction becomes a
    dependency of it: preps are always emitted right before the first
    conv stretch that needs them, never earlier.
  - The bass2jax/pseudo-DMA path allows only ONE sync wait per DMA; every
    DMA destination is a fresh tile (or a disjoint slice of one), so no
    DMA ever needs more than one semaphore wait. All loads are issued
    up-front in priority order (DMA transfers serialize): image-0 pieces
    interleaved with the split w1 consts, then halves of images 1..3.
    Stores go out per (image, chunk), per-stretch for the last image.
  - Image 0 is loaded/prepped/conv'd in 7-row pieces so the first conv
    matmuls start ~4.9us in; per-row matmul waits keep the rest of the
    pipeline gapless (PE runs ~94us continuously).
  - A short burst of junk transposes warms the PE p-state ramp so the
    first real conv matmuls run at full clock.
"""

import os
import numpy as np

os.environ.setdefault("MYCRO_LOCAL_CACHE", "1")

N_CORES = 8
C = 256
NCHUNK = 2  # channel chunks of 128
KP = 128

# packed consts layout (bytes per partition); w1 split by output-channel
# chunk so the first conv can start as soon as the j0 half lands
W1J0_OFF = 0        # fp8 [9,2,128] -> 2304 B
NT1_OFF = 2304      # f32 [2] -> 8 B
CBA_B = 2312        # first consts DMA covers [0, CBA_B)
W1J1_OFF = 2312     # fp8 -> 2304 B
CBB_B = 4616        # second consts DMA covers [CBA_B, CBB_B)
W2_OFF = 4616       # fp8 [9,2,2,128] -> 4608 B
S2_OFF = 9224       # f32 [2]
B2_OFF = 9232       # f32 [2]
CONST_B = 9240


def build_program(B, H, W, psum_stretch=1024, conv_bufs=4, warm_mm=46, tail_split=0, exact_rows=True):
    """Build the per-core Bass program. B images of HxWxC per core."""
    import concourse.bass as bass
    import concourse.bacc as bacc
    import concourse.tile as tile
    from concourse import mybir

    F32 = mybir.dt.float32
    FP8 = mybir.dt.float8e4
    BF16 = mybir.dt.bfloat16
    U8 = mybir.dt.uint8
    DR = mybir.MatmulPerfMode.DoubleRow
    Alu = mybir.AluOpType
    Act = mybir.ActivationFunctionType

    Hp, Wp = H + 2, W + 2
    S_pad = Hp * Wp
    S = H * W
    DOFF = 32  # left zero pad inside each channel-chunk row buffer
    S_chunk = ((S_pad + DOFF + 32 + 15) // 16) * 16  # right pad >= 32
    NQ = 4  # prep groups (and img-0 load quarters) per image
    GR = H // NQ  # rows per prep group
    assert H % NQ == 0
    PO = (H // 2) * (W // 2)
    WH = W // 2

    # conv2 row groups (pool-pair aligned)
    max_rows = (psum_stretch // Wp) // 2 * 2
    row_groups = []
    r = 0
    while r < H:
        g = min(max_rows, H - r)
        row_groups.append((r, g))
        r += g
    st2 = [((1 + r0) * Wp, rg * Wp, r0, rg) for r0, rg in row_groups]
    # last image: split the final row group so the pool/store chain after
    # the very last matmul is as short as possible
    tail_rows = list(row_groups[:-1])
    lr0, lrg = row_groups[-1]
    if tail_split and lrg > tail_split:
        tail_rows += [(lr0, lrg - tail_split), (lr0 + lrg - tail_split, tail_split)]
    else:
        tail_rows.append((lr0, lrg))
    st2_tail = [((1 + r0) * Wp, rg * Wp, r0, rg) for r0, rg in tail_rows]
    # image 0 is prepped in 7-row pieces (NP1 of them) with conv1 stretches
    # aligned so stretch s only reads input rows loaded by pieces <= s. The
    # +Wp+1 shifted window spills one byte into the row after r0+rg, so
    # reserve one extra row per boundary.
    # pieces: two 7-row halves of the first quarter, then whole quarters.
    # stretch s may read up to one row past its end, so each stretch stops
    # two rows short of its piece's cumulative coverage.
    GRH = GR // 2
    p0 = [(k * GRH, GRH) for k in range(4)] + [
        (GR * k, GR) for k in range(2, NQ)
    ]
    rg1 = []
    r = 0
    cum = 0
    for i, (plo, pn) in enumerate(p0):
        cum += pn
        hi = H if i == len(p0) - 1 else cum - 2
        rg1.append((r, hi - r))
        r = hi
    NP1 = len(p0)
    st1_first = [((1 + r0) * Wp, rg * Wp, r0, rg) for r0, rg in rg1]
    st1_rest = st2
    PS_COLS = psum_stretch

    nc = bacc.Bacc("TRN2", target_bir_lowering=False, debug=False)

    x_h = nc.dram_tensor("x", [B, C, S], F32, kind="ExternalInput")
    cb_h = nc.dram_tensor("cb", [KP, CONST_B], U8, kind="ExternalInput")
    y_h = nc.dram_tensor("y", [B, NCHUNK, KP, PO], F32, kind="ExternalOutput")

    def dram_ap(handle, offset, dims):
        return bass.AP(
            tensor=handle.ap().tensor, offset=offset, ap=[list(d) for d in dims]
        )

    with tile.TileContext(nc) as tc:
        from contextlib import ExitStack

        with ExitStack() as ctx:
            consts = ctx.enter_context(tc.tile_pool(name="consts", bufs=1))
            xnat_p = ctx.enter_context(tc.tile_pool(name="xnat", bufs=1))
            xsT_p = ctx.enter_context(tc.tile_pool(name="xsT", bufs=2))
            hsT_p = ctx.enter_context(tc.tile_pool(name="hsT", bufs=2))
            pr_p = ctx.enter_context(tc.tile_pool(name="prp", bufs=2))
            po_p = ctx.enter_context(tc.tile_pool(name="pop", bufs=2))
            convp = ctx.enter_context(
                tc.tile_pool(name="convp", bufs=conv_bufs, space="PSUM")
            )

            # --- packed constants (three DMAs: w1-j0+nt1, w1-j1, rest)
            cb = consts.tile([KP, CONST_B], U8)
            w1j = [
                cb[:, W1J0_OFF : W1J0_OFF + 2304].bitcast(FP8).rearrange(
                    "p (t k m) -> p t k m", t=9, k=2
                ),
                cb[:, W1J1_OFF : W1J1_OFF + 2304].bitcast(FP8).rearrange(
                    "p (t k m) -> p t k m", t=9, k=2
                ),
            ]
            w2sb = cb[:, W2_OFF : W2_OFF + 4608].bitcast(FP8).rearrange(
                "p (t j k m) -> p t j k m", t=9, j=NCHUNK, k=2
            )
            w1v = lambda j, t: w1j[j][:, t]
            w2v = lambda j, t: w2sb[:, t, j]
            nt1sb = cb[:, NT1_OFF : NT1_OFF + 8].bitcast(F32)
            s2sb = cb[:, S2_OFF : S2_OFF + 8].bitcast(F32)
            b2sb = cb[:, B2_OFF : B2_OFF + 8].bitcast(F32)

            # --- preload the ACT piecewise-poly table (Sign) with a tiny
            # dependency-free activation so the 1.3us table load is off the
            # critical prep chain
            dummy = consts.tile([1, 4], F32)
            nc.vector.memset(dummy, 0.0)
            nc.scalar.activation(dummy, dummy, Act.Sign, bias=0.0, scale=1.0)

            from concourse import masks

            id8sb = consts.tile([KP, KP], BF16)
            masks.make_identity(nc, id8sb)

            # --- PE p-state warmup: dependency-free junk transposes keep the
            # tensor engine busy from t~0 so the ramp is spent before real
            # conv matmuls arrive. The junk lives in a convp rotation buffer
            # (conv matmuls start=True overwrite it later).
            if warm_mm:
                warm = convp.tile([KP, KP], BF16, tag="cv", name="warm")
                for _ in range(warm_mm):
                    nc.tensor.transpose(warm, id8sb, id8sb)

            # --- loads, issued in priority order (DMA transfers serialize)
            xn = {}
            for img in range(B):
                xn[img] = xnat_p.tile(
                    [KP, NCHUNK, S], F32, tag=f"xn{img}", name=f"xn{img}"
                )

            def load_x_span(img, s0, s1):
                nc.sync.dma_start(
                    out=xn[img][:, :, s0:s1],
                    in_=dram_ap(
                        x_h,
                        img * C * S + s0,
                        [[S, KP], [KP * S, NCHUNK], [1, s1 - s0]],
                    ),
                )

            Q = GR * W   # spatial elems per steady-state prep quarter
            load_x_span(0, 0, GRH * W)
            nc.sync.dma_start(out=cb[:, :CBA_B], in_=cb_h.ap()[:, :CBA_B])
            load_x_span(0, GRH * W, GR * W)
            nc.sync.dma_start(out=cb[:, CBA_B:CBB_B], in_=cb_h.ap()[:, CBA_B:CBB_B])
            for plo, pn in p0[2:]:
                load_x_span(0, plo * W, (plo + pn) * W)
            nc.sync.dma_start(out=cb[:, CBB_B:], in_=cb_h.ap()[:, CBB_B:])
            for img in range(1, B):
                load_x_span(img, 0, S // 2)
                load_x_span(img, S // 2, S)

            # --- helpers
            def border_memsets(buf):
                # rows 0 and H+1, left/right pads, and border cols {0, W+1} of
                # rows 1..H; on GPSIMD so the vector engines stay free.
                nc.gpsimd.memset(buf[:, :, 0 : DOFF + Wp], 0.0)
                nc.gpsimd.memset(buf[:, :, DOFF + (H + 1) * Wp : S_chunk], 0.0)
                rows = buf[:, :, DOFF + Wp : DOFF + (H + 1) * Wp].rearrange(
                    "p j (r w) -> p j r w", w=Wp
                )
                nc.gpsimd.memset(rows[:, :, :, 0 :: (W + 1)], 0.0)

            xsT_tiles = {}

            def prep_span(img, lo, hi, all_dve=False):
                # fused sign+scatter of rows [lo, hi): fp32 -> fp8 +-0.5
                # written straight into the padded conv layout; j0 on DVE,
                # j1 on GPSIMD so the two planes run in parallel (image 0:
                # both on DVE, whose op is 2.4x faster than GPSIMD's)
                if lo == 0:
                    xsT_tiles[img] = xsT_p.tile(
                        [KP, NCHUNK, S_chunk], FP8, tag="xsT", name=f"xsT{img}"
                    )
                    border_memsets(xsT_tiles[img])
                xsT = xsT_tiles[img]
                a0 = DOFF + (1 + lo) * Wp
                for j in range(NCHUNK):
                    src = xn[img][:, j, lo * W : hi * W].rearrange(
                        "p (r w) -> p r w", w=W
                    )
                    dst = xsT[:, j, a0 : a0 + (hi - lo) * Wp].rearrange(
                        "p (r w) -> p r w", w=Wp
                    )[:, :, 1 : 1 + W]
                    eng = nc.vector if (j == 0 or all_dve) else nc.gpsimd
                    eng.tensor_scalar(
                        dst, src, 0.0, 0.5, Alu.is_ge, Alu.subtract
                    )

            def prep_group(img, g):
                prep_span(img, g * GR, (g + 1) * GR)

            def conv_stretch(inbuf, wv, st, si, j, psum_cb, nm):
                cs, cn, r0, rg = st
                ps = convp.tile([KP, PS_COLS], F32, tag="cv", name=f"cv{nm}{si}{j}")
                if exact_rows:
                    # per-row 56-col matmuls (skip the 2 pad cols per row),
                    # row-outer/tap-inner so each row's PSUM accumulation
                    # group opens and closes before the next row touches the
                    # same 2KB bank. Rows sit at a 64-col pitch so no matmul
                    # output straddles a bank.
                    for r in range(rg):
                        for t in range(9):
                            dy, dx = t // 3, t % 3
                            a = DOFF + (r0 + r + dy) * Wp + dx
                            nc.tensor.matmul(
                                ps[:, r * 64 : r * 64 + W],
                                wv(j, t),
                                inbuf[:, :, a : a + W],
                                start=(t == 0),
                                stop=(t == 8),
                                perf_mode=DR,
                            )
                else:
                    for t in range(9):
                        dy, dx = t // 3, t % 3
                        lhsT = wv(j, t)
                        off = (dy - 1) * Wp + (dx - 1)
                        for c0 in range(0, cn, 512):
                            n = min(512, cn - c0)
                            a = DOFF + cs + off + c0
                            nc.tensor.matmul(
                                ps[:, c0 : c0 + n],
                                lhsT,
                                inbuf[:, :, a : a + n],
                                start=(t == 0),
                                stop=(t == 8),
                                perf_mode=DR,
                            )
                psum_cb(si, j, ps, st)

            hsT_tiles = {}

            def conv1_stretch(img, si):
                sts1 = st1_first if img == 0 else st1_rest
                st = sts1[si]
                if si == 0:
                    hsT_tiles[img] = hsT_p.tile(
                        [KP, NCHUNK, S_chunk], FP8, tag="hsT", name=f"hsT{img}"
                    )
                    border_memsets(hsT_tiles[img])
                hsT = hsT_tiles[img]

                def bnsign(si_, j, ps, st_):
                    cs, cn, r0_, rg_ = st_
                    dstv = hsT[:, j, DOFF + cs : DOFF + cs + cn].rearrange(
                        "p (r w) -> p r w", w=Wp
                    )[:, :, 1 : 1 + W]
                    if exact_rows:
                        srcv = ps[:, : rg_ * 64].rearrange("p (r w) -> p r w", w=64)[
                            :, :, :W
                        ]
                    else:
                        srcv = ps[:, :cn].rearrange("p (r w) -> p r w", w=Wp)[
                            :, :, 1 : 1 + W
                        ]
                    nc.scalar.activation(
                        dstv, srcv, Act.Sign, bias=nt1sb[:, j : j + 1], scale=1.0
                    )

                for j in range(NCHUNK):
                    conv_stretch(xsT_tiles[img], w1v, st, si, j, bnsign, f"a{img}")
                if si == len(sts1) - 1:
                    xsT_tiles.pop(img)

            pr_tiles = {}
            pooled_tiles = {}

            def conv2_stretch(img, si):
                sts = st2_tail if img == B - 1 else st2
                st = sts[si]
                if si == 0:
                    pr_tiles[img] = [
                        pr_p.tile([KP, H // 2, W], F32, tag="pr", name=f"pr{img}{j}")
                        for j in range(NCHUNK)
                    ]
                    pooled_tiles[img] = [
                        po_p.tile([KP, PO], F32, tag="pooled", name=f"pl{img}{j}")
                        for j in range(NCHUNK)
                    ]

                def pool_cb(si_, j, ps, st_):
                    cs, cn, r0, rg = st_
                    if exact_rows:
                        rows = ps[:, : rg * 64].rearrange("p (q t) -> p q t", t=128)
                        in0 = rows[:, :, 0:W]
                        in1 = rows[:, :, 64 : 64 + W]
                    else:
                        rows = ps[:, : rg * Wp].rearrange("p (q t) -> p q t", t=2 * Wp)
                        in0 = rows[:, :, 1 : 1 + W]
                        in1 = rows[:, :, Wp + 1 : Wp + 1 + W]
                    q0, q1 = r0 // 2, (r0 + rg) // 2
                    q = rg // 2
                    # TensorTensor may read only one input from PSUM: stage
                    # the even rows into SBUF, then max against the PSUM odd
                    # rows. Steady state splits the copy to ACT for engine
                    # parallelism; the last image's short tail chains run
                    # entirely on DVE to avoid cross-engine sem hops.
                    prA = pr_p.tile(
                        [KP, max_rows // 2, W], F32, tag="prA", bufs=4,
                        name=f"prA{img}{si_}{j}",
                    )
                    nc.scalar.copy(prA[:, :q, :], in0)
                    nc.vector.tensor_max(
                        pr_tiles[img][j][:, q0:q1, :], prA[:, :q, :], in1
                    )
                    prs = pr_tiles[img][j][:, q0:q1, :].rearrange("p q w -> p (q w)")
                    pv = pooled_tiles[img][j].rearrange("p (q w) -> p q w", w=WH)[
                        :, q0:q1, :
                    ]
                    nc.vector.tensor_max(pv, prs[:, 0::2], prs[:, 1::2])
                    nc.vector.tensor_scalar(
                        pv, pv, s2sb[:, j : j + 1], b2sb[:, j : j + 1],
                        Alu.mult, Alu.add,
                    )
                    # stores: whole channel-chunk per image, but per-stretch
                    # for the last image so the tail ships immediately
                    if img == B - 1:
                        nc.sync.dma_start(
                            out=dram_ap(
                                y_h,
                                (img * NCHUNK + j) * KP * PO + q0 * WH,
                                [[PO, KP], [1, (q1 - q0) * WH]],
                            ),
                            in_=pooled_tiles[img][j][:, q0 * WH : q1 * WH],
                        )
                    elif si_ == len(sts) - 1:
                        nc.sync.dma_start(
                            out=dram_ap(
                                y_h,
                                (img * NCHUNK + j) * KP * PO,
                                [[PO, KP], [1, PO]],
                            ),
                            in_=pooled_tiles[img][j],
                        )

                for j in range(NCHUNK):
                    conv_stretch(hsT_tiles[img], w2v, st, si, j, pool_cb, f"b{img}")
                if si == len(sts) - 1:
                    hsT_tiles.pop(img)

            # --- emission ---
            # image 0: prep each 7-row piece right before the conv1 stretch
            # that needs it. The conv rhs spans both channel planes as one
            # interval hull, so any copy emitted before a stretch becomes a
            # dependency of it — never emit a copy ahead of an earlier
            # stretch.
            for k in range(NP1):
                plo, pn = p0[k]
                prep_span(0, plo, plo + pn, all_dve=True)
                conv1_stretch(0, k)
            for img in range(B):
                if img > 0:
                    for si in range(len(st2)):
                        conv1_stretch(img, si)
                        if img + 1 < B:
                            prep_group(img + 1, si)
                for si in range(len(st2_tail if img == B - 1 else st2)):
                    if img == 0 and B > 1 and si < NQ:
                        prep_group(1, si)
                    conv2_stretch(img, si)

    nc.compile()
    return nc


# ---------------------------------------------------------------------------
# host-side constant prep
# ---------------------------------------------------------------------------


def _prep_consts(w1, beta1, mean1, var1, w2, beta2, mean2, var2):
    import jax
    import jax.numpy as jnp
    from jax import lax
    from concourse import mybir

    fp8np = mybir.dt.np(mybir.dt.float8e4)

    def prep_w(w, j_major=False):
        ws = np.where(np.asarray(w) >= 0, np.float32(1.0), np.float32(-1.0))
        # [3,3,ci,co] -> [p, (j,) tap, ktile, m]; ci = ktile*128+p, co = j*128+m
        wr = ws.reshape(9, 2, KP, NCHUNK, KP)
        wr = wr.transpose((2, 3, 0, 1, 4) if j_major else (2, 0, 3, 1, 4))
        return np.ascontiguousarray(wr).astype(fp8np)

    w1p, w2p = prep_w(w1, j_major=True), prep_w(w2)

    cpu = jax.devices("cpu")[0]
    MAXH = 9 * C
    with jax.default_device(cpu):
        hs = jnp.arange(-MAXH, MAXH + 1, dtype=jnp.float32)
        bn1 = (hs[:, None] - jnp.asarray(mean1)[None, :]) * lax.rsqrt(
            jnp.asarray(var1) + 1e-3
        )[None, :] + jnp.asarray(beta1)[None, :]
        nonneg = np.asarray(bn1 >= 0)
        r2 = np.asarray(lax.rsqrt(jnp.asarray(var2) + 1e-3))

    assert (np.diff(nonneg.astype(np.int8), axis=0) >= 0).all(), "bn1 not monotone"
    kc = np.where(nonneg.any(0), nonneg.argmax(0), 2 * MAXH + 1) - MAXH
    # device psum holds h/2 (x=+-0.5, w=+-1): sign flips at (kc-0.5)/2
    nt1 = (-(kc.astype(np.float64) - 0.5) / 2.0).astype(np.float32)

    s2 = r2.astype(np.float32)
    b2 = (
        np.asarray(beta2, np.float64)
        - np.asarray(mean2, np.float64) * s2.astype(np.float64)
    ).astype(np.float32)

    def to_pj(a):  # [256] -> [128, 2] with c = j*128+p
        return np.ascontiguousarray(a.reshape(NCHUNK, KP).T).astype(np.float32)

    # pack everything into one [128, CONST_B] uint8 image
    cbuf = np.zeros((KP, CONST_B), dtype=np.uint8)

    def put(off, arr):
        by = np.ascontiguousarray(arr).reshape(KP, -1).view(np.uint8)
        cbuf[:, off : off + by.shape[1]] = by

    put(W1J0_OFF, w1p[:, 0])
    put(W1J1_OFF, w1p[:, 1])
    put(W2_OFF, w2p)
    put(NT1_OFF, to_pj(nt1))
    put(S2_OFF, to_pj(s2))
    put(B2_OFF, to_pj(b2))
    return {"cb": cbuf}


# ---------------------------------------------------------------------------
# entry point
# ---------------------------------------------------------------------------

_cached = {}


def _run(inputs, trace=False):
    from concourse import bass_utils

    x = np.asarray(inputs["x"], dtype=np.float32)
    Bt, H, W, _ = x.shape  # 32, 56, 56, 256
    Bc = Bt // N_CORES

    consts = _prep_consts(
        inputs["w1"], inputs["beta1"], inputs["mean1"], inputs["var1"],
        inputs["w2"], inputs["beta2"], inputs["mean2"], inputs["var2"],
    )

    key = (Bc, H, W)
    if key not in _cached:
        _cached[key] = build_program(Bc, H, W)
    nc = _cached[key]

    # channel-major marshaling: [Bt, H*W, C] -> per-core [Bc, C, H*W]
    xcm = np.ascontiguousarray(
        x.reshape(Bt, H * W, C).transpose(0, 2, 1)
    )

    in_maps = []
    for c in range(N_CORES):
        m = dict(consts)
        m["x"] = xcm[c * Bc : (c + 1) * Bc]
        in_maps.append(m)

    res = bass_utils.run_bass_kernel_spmd(
        nc, in_maps, core_ids=list(range(N_CORES)), trace=trace
    )
    PO = (H // 2) * (W // 2)
    # y comes back channel-major [Bc, 2, 128, PO]; restore NHWC
    y = np.concatenate(
        [r["y"].reshape(Bc, C, PO).transpose(0, 2, 1) for r in res.results], axis=0
    )
    y = np.ascontiguousarray(y.reshape(Bt, H // 2, W // 2, C)).astype(np.float32)
    return y, res


def kernel(**inputs):
    y, _ = _run(inputs, trace=False)
    return y


# revision 71
# speedup vs baseline: 1.0667x; 1.0667x over previous
"""Trainium2 Bass kernel for a BinaryNet conv block.

Pipeline (per core, data-parallel over batch):
  sign(x) -> conv3x3(sign(w1)) -> BN1 -> sign -> conv3x3(sign(w2))
          -> maxpool2x2 -> BN2

Implementation notes:
  - Activations are +-0.5, weights +-1.0 in fp8e4 (exactly representable);
    convs run as per-row matmuls tiled 50+6 cols with DoubleRow perf mode
    (K=256 contraction per instruction), accumulating exactly into fp32
    PSUM. Per-row emission skips the 2 pad columns per row; rows sit at a
    64-col PSUM pitch so no matmul output straddles a 2KB bank and each
    piece's accumulation group opens/closes before the next touches the
    bank (row-outer, piece-outer, tap-inner).
  - BN1+sign is fused into one ScalarE Sign activation against a
    host-precomputed per-channel threshold. Conv outputs are exact
    integers, so an integer cutoff k_c reproduces the reference's fp32
    sign decisions bit-exactly.
  - The host marshals x to channel-major [C, H*W] per image and reads y
    back channel-major [2, 128, PO]; the device never transposes. The PE
    therefore runs conv matmuls only, fed by a fused sign+scatter
    tensor_scalar (fp32 -> fp8 +-0.5, j0 on DVE, j1 on GPSIMD) writing
    straight into the zero-bordered padded layout.
  - Spatial layout is channel-major [ci, y*(W+2)+x] with a zero border so
    the 9 taps are just constant AP offsets.
  - Dependency tracking treats a multi-plane read as one interval hull,
    so a j0-plane prep write emitted before a conv instruction becomes a
    dependency of it: preps are always emitted right before the first
    conv stretch that needs them, never earlier.
  - The bass2jax/pseudo-DMA path allows only ONE sync wait per DMA; every
    DMA destination is a fresh tile (or a disjoint slice of one), so no
    DMA ever needs more than one semaphore wait. All loads are issued
    up-front in priority order (DMA transfers serialize): image-0 pieces
    interleaved with the split w1 consts, then halves of images 1..3.
    Stores go out per (image, chunk), per-stretch for the last image.
  - Image 0 is loaded/prepped/conv'd in 7-row pieces so the first conv
    matmuls start ~4.9us in; per-row matmul waits keep the rest of the
    pipeline gapless (PE runs ~94us continuously).
  - A short burst of junk transposes warms the PE p-state ramp so the
    first real conv matmuls run at full clock.
"""

import os
import numpy as np

os.environ.setdefault("MYCRO_LOCAL_CACHE", "1")

N_CORES = 8
C = 256
NCHUNK = 2  # channel chunks of 128
KP = 128

# packed consts layout (bytes per partition); w1 split by output-channel
# chunk so the first conv can start as soon as the j0 half lands
W1J0_OFF = 0        # fp8 [9,2,128] -> 2304 B
NT1_OFF = 2304      # f32 [2] -> 8 B
CBA_B = 2312        # first consts DMA covers [0, CBA_B)
W1J1_OFF = 2312     # fp8 -> 2304 B
CBB_B = 4616        # second consts DMA covers [CBA_B, CBB_B)
W2_OFF = 4616       # fp8 [9,2,2,128] -> 4608 B
S2_OFF = 9224       # f32 [2]
B2_OFF = 9232       # f32 [2]
CONST_B = 9240


def build_program(B, H, W, psum_stretch=1024, conv_bufs=4, warm_mm=46, tail_split=0, exact_rows=True):
    """Build the per-core Bass program. B images of HxWxC per core."""
    import concourse.bass as bass
    import concourse.bacc as bacc
    import concourse.tile as tile
    from concourse import mybir

    F32 = mybir.dt.float32
    FP8 = mybir.dt.float8e4
    BF16 = mybir.dt.bfloat16
    U8 = mybir.dt.uint8
    DR = mybir.MatmulPerfMode.DoubleRow
    Alu = mybir.AluOpType
    Act = mybir.ActivationFunctionType

    Hp, Wp = H + 2, W + 2
    S_pad = Hp * Wp
    S = H * W
    DOFF = 32  # left zero pad inside each channel-chunk row buffer
    S_chunk = ((S_pad + DOFF + 32 + 15) // 16) * 16  # right pad >= 32
    NQ = 4  # prep groups (and img-0 load quarters) per image
    GR = H // NQ  # rows per prep group
    assert H % NQ == 0
    PO = (H // 2) * (W // 2)
    WH = W // 2

    # conv2 row groups (pool-pair aligned)
    max_rows = (psum_stretch // Wp) // 2 * 2
    row_groups = []
    r = 0
    while r < H:
        g = min(max_rows, H - r)
        row_groups.append((r, g))
        r += g
    st2 = [((1 + r0) * Wp, rg * Wp, r0, rg) for r0, rg in row_groups]
    # last image: split the final row group so the pool/store chain after
    # the very last matmul is as short as possible
    tail_rows = list(row_groups[:-1])
    lr0, lrg = row_groups[-1]
    if tail_split and lrg > tail_split:
        tail_rows += [(lr0, lrg - tail_split), (lr0 + lrg - tail_split, tail_split)]
    else:
        tail_rows.append((lr0, lrg))
    st2_tail = [((1 + r0) * Wp, rg * Wp, r0, rg) for r0, rg in tail_rows]
    # image 0 is prepped in 7-row pieces (NP1 of them) with conv1 stretches
    # aligned so stretch s only reads input rows loaded by pieces <= s. The
    # +Wp+1 shifted window spills one byte into the row after r0+rg, so
    # reserve one extra row per boundary.
    # pieces: two 7-row halves of the first quarter, then whole quarters.
    # stretch s may read up to one row past its end, so each stretch stops
    # two rows short of its piece's cumulative coverage.
    GRH = GR // 2
    p0 = [(k * GRH, GRH) for k in range(2 * NQ)]
    rg1 = []
    r = 0
    cum = 0
    for i, (plo, pn) in enumerate(p0):
        cum += pn
        hi = H if i == len(p0) - 1 else cum - 2
        while hi - r > max_rows:
            rg1.append((r, max_rows))
            r += max_rows
        rg1.append((r, hi - r))
        r = hi
    NP1 = len(p0)
    st1_first = [((1 + r0) * Wp, rg * Wp, r0, rg) for r0, rg in rg1]
    st1_rest = st2
    PS_COLS = psum_stretch

    nc = bacc.Bacc("TRN2", target_bir_lowering=False, debug=False)

    x_h = nc.dram_tensor("x", [B, C, S], F32, kind="ExternalInput")
    cb_h = nc.dram_tensor("cb", [KP, CONST_B], U8, kind="ExternalInput")
    y_h = nc.dram_tensor("y", [B, NCHUNK, KP, PO], F32, kind="ExternalOutput")

    def dram_ap(handle, offset, dims):
        return bass.AP(
            tensor=handle.ap().tensor, offset=offset, ap=[list(d) for d in dims]
        )

    with tile.TileContext(nc) as tc:
        from contextlib import ExitStack

        with ExitStack() as ctx:
            consts = ctx.enter_context(tc.tile_pool(name="consts", bufs=1))
            xnat_p = ctx.enter_context(tc.tile_pool(name="xnat", bufs=1))
            xsT_p = ctx.enter_context(tc.tile_pool(name="xsT", bufs=2))
            hsT_p = ctx.enter_context(tc.tile_pool(name="hsT", bufs=2))
            pr_p = ctx.enter_context(tc.tile_pool(name="prp", bufs=2))
            po_p = ctx.enter_context(tc.tile_pool(name="pop", bufs=2))
            convp = ctx.enter_context(
                tc.tile_pool(name="convp", bufs=conv_bufs, space="PSUM")
            )

            # --- packed constants (three DMAs: w1-j0+nt1, w1-j1, rest)
            cb = consts.tile([KP, CONST_B], U8)
            w1j = [
                cb[:, W1J0_OFF : W1J0_OFF + 2304].bitcast(FP8).rearrange(
                    "p (t k m) -> p t k m", t=9, k=2
                ),
                cb[:, W1J1_OFF : W1J1_OFF + 2304].bitcast(FP8).rearrange(
                    "p (t k m) -> p t k m", t=9, k=2
                ),
            ]
            w2sb = cb[:, W2_OFF : W2_OFF + 4608].bitcast(FP8).rearrange(
                "p (t j k m) -> p t j k m", t=9, j=NCHUNK, k=2
            )
            w1v = lambda j, t: w1j[j][:, t]
            w2v = lambda j, t: w2sb[:, t, j]
            nt1sb = cb[:, NT1_OFF : NT1_OFF + 8].bitcast(F32)
            s2sb = cb[:, S2_OFF : S2_OFF + 8].bitcast(F32)
            b2sb = cb[:, B2_OFF : B2_OFF + 8].bitcast(F32)

            # --- preload the ACT piecewise-poly table (Sign) with a tiny
            # dependency-free activation so the 1.3us table load is off the
            # critical prep chain
            dummy = consts.tile([1, 4], F32)
            nc.vector.memset(dummy, 0.0)
            nc.scalar.activation(dummy, dummy, Act.Sign, bias=0.0, scale=1.0)

            from concourse import masks

            id8sb = consts.tile([KP, KP], BF16)
            masks.make_identity(nc, id8sb)

            # --- PE p-state warmup: dependency-free junk transposes keep the
            # tensor engine busy from t~0 so the ramp is spent before real
            # conv matmuls arrive. The junk lives in a convp rotation buffer
            # (conv matmuls start=True overwrite it later).
            if warm_mm:
                warm = convp.tile([KP, KP], BF16, tag="cv", name="warm")
                for _ in range(warm_mm):
                    nc.tensor.transpose(warm, id8sb, id8sb)

            # --- loads, issued in priority order (DMA transfers serialize)
            xn = {}
            for img in range(B):
                xn[img] = xnat_p.tile(
                    [KP, NCHUNK, S], F32, tag=f"xn{img}", name=f"xn{img}"
                )

            def load_x_span(img, s0, s1):
                nc.sync.dma_start(
                    out=xn[img][:, :, s0:s1],
                    in_=dram_ap(
                        x_h,
                        img * C * S + s0,
                        [[S, KP], [KP * S, NCHUNK], [1, s1 - s0]],
                    ),
                )

            Q = GR * W   # spatial elems per steady-state prep quarter
            load_x_span(0, 0, GRH * W)
            nc.sync.dma_start(out=cb[:, :CBA_B], in_=cb_h.ap()[:, :CBA_B])
            load_x_span(0, GRH * W, GR * W)
            nc.sync.dma_start(out=cb[:, CBA_B:CBB_B], in_=cb_h.ap()[:, CBA_B:CBB_B])
            for plo, pn in p0[2:]:
                load_x_span(0, plo * W, (plo + pn) * W)
            nc.sync.dma_start(out=cb[:, CBB_B:], in_=cb_h.ap()[:, CBB_B:])
            for img in range(1, B):
                load_x_span(img, 0, S // 2)
                load_x_span(img, S // 2, S)

            # --- helpers
            def border_memsets(buf):
                # rows 0 and H+1, left/right pads, and border cols {0, W+1} of
                # rows 1..H; on GPSIMD so the vector engines stay free.
                nc.gpsimd.memset(buf[:, :, 0 : DOFF + Wp], 0.0)
                nc.gpsimd.memset(buf[:, :, DOFF + (H + 1) * Wp : S_chunk], 0.0)
                rows = buf[:, :, DOFF + Wp : DOFF + (H + 1) * Wp].rearrange(
                    "p j (r w) -> p j r w", w=Wp
                )
                nc.gpsimd.memset(rows[:, :, :, 0 :: (W + 1)], 0.0)

            xsT_tiles = {}

            def prep_span(img, lo, hi, all_dve=False):
                # fused sign+scatter of rows [lo, hi): fp32 -> fp8 +-0.5
                # written straight into the padded conv layout; j0 on DVE,
                # j1 on GPSIMD so the two planes run in parallel (image 0:
                # both on DVE, whose op is 2.4x faster than GPSIMD's)
                if lo == 0:
                    xsT_tiles[img] = xsT_p.tile(
                        [KP, NCHUNK, S_chunk], FP8, tag="xsT", name=f"xsT{img}"
                    )
                    border_memsets(xsT_tiles[img])
                xsT = xsT_tiles[img]
                a0 = DOFF + (1 + lo) * Wp
                for j in range(NCHUNK):
                    src = xn[img][:, j, lo * W : hi * W].rearrange(
                        "p (r w) -> p r w", w=W
                    )
                    dst = xsT[:, j, a0 : a0 + (hi - lo) * Wp].rearrange(
                        "p (r w) -> p r w", w=Wp
                    )[:, :, 1 : 1 + W]
                    eng = nc.vector if (j == 0 or all_dve) else nc.gpsimd
                    eng.tensor_scalar(
                        dst, src, 0.0, 0.5, Alu.is_ge, Alu.subtract
                    )

            def prep_group(img, g):
                prep_span(img, g * GR, (g + 1) * GR)

            def conv_stretch(inbuf, wv, st, si, j, psum_cb, nm):
                cs, cn, r0, rg = st
                ps = convp.tile([KP, PS_COLS], F32, tag="cv", name=f"cv{nm}{si}{j}")
                if exact_rows:
                    # per-row matmuls (skip the 2 pad cols per row), split
                    # 50+6 cols per row. Row-outer, piece-outer, tap-inner:
                    # each piece's PSUM accumulation group opens and closes
                    # before the next touches the same 2KB bank. Rows sit at
                    # a 64-col pitch so no matmul output straddles a bank.
                    for r in range(rg):
                        for c0, cw in ((0, 50), (50, W - 50)):
                            for t in range(9):
                                dy, dx = t // 3, t % 3
                                a = DOFF + (r0 + r + dy) * Wp + dx + c0
                                nc.tensor.matmul(
                                    ps[:, r * 64 + c0 : r * 64 + c0 + cw],
                                    wv(j, t),
                                    inbuf[:, :, a : a + cw],
                                    start=(t == 0),
                                    stop=(t == 8),
                                    perf_mode=DR,
                                )
                else:
                    for t in range(9):
                        dy, dx = t // 3, t % 3
                        lhsT = wv(j, t)
                        off = (dy - 1) * Wp + (dx - 1)
                        for c0 in range(0, cn, 512):
                            n = min(512, cn - c0)
                            a = DOFF + cs + off + c0
                            nc.tensor.matmul(
                                ps[:, c0 : c0 + n],
                                lhsT,
                                inbuf[:, :, a : a + n],
                                start=(t == 0),
                                stop=(t == 8),
                                perf_mode=DR,
                            )
                psum_cb(si, j, ps, st)

            hsT_tiles = {}

            def conv1_stretch(img, si):
                sts1 = st1_first if img == 0 else st1_rest
                st = sts1[si]
                if si == 0:
                    hsT_tiles[img] = hsT_p.tile(
                        [KP, NCHUNK, S_chunk], FP8, tag="hsT", name=f"hsT{img}"
                    )
                    border_memsets(hsT_tiles[img])
                hsT = hsT_tiles[img]

                def bnsign(si_, j, ps, st_):
                    cs, cn, r0_, rg_ = st_
                    dstv = hsT[:, j, DOFF + cs : DOFF + cs + cn].rearrange(
                        "p (r w) -> p r w", w=Wp
                    )[:, :, 1 : 1 + W]
                    if exact_rows:
                        srcv = ps[:, : rg_ * 64].rearrange("p (r w) -> p r w", w=64)[
                            :, :, :W
                        ]
                    else:
                        srcv = ps[:, :cn].rearrange("p (r w) -> p r w", w=Wp)[
                            :, :, 1 : 1 + W
                        ]
                    nc.scalar.activation(
                        dstv, srcv, Act.Sign, bias=nt1sb[:, j : j + 1], scale=1.0
                    )

                for j in range(NCHUNK):
                    conv_stretch(xsT_tiles[img], w1v, st, si, j, bnsign, f"a{img}")
                if si == len(sts1) - 1:
                    xsT_tiles.pop(img)

            pr_tiles = {}
            pooled_tiles = {}

            def conv2_stretch(img, si):
                sts = st2_tail if img == B - 1 else st2
                st = sts[si]
                if si == 0:
                    pr_tiles[img] = [
                        pr_p.tile([KP, H // 2, W], F32, tag="pr", name=f"pr{img}{j}")
                        for j in range(NCHUNK)
                    ]
                    pooled_tiles[img] = [
                        po_p.tile([KP, PO], F32, tag="pooled", name=f"pl{img}{j}")
                        for j in range(NCHUNK)
                    ]

                def pool_cb(si_, j, ps, st_):
                    cs, cn, r0, rg = st_
                    if exact_rows:
                        rows = ps[:, : rg * 64].rearrange("p (q t) -> p q t", t=128)
                        in0 = rows[:, :, 0:W]
                        in1 = rows[:, :, 64 : 64 + W]
                    else:
                        rows = ps[:, : rg * Wp].rearrange("p (q t) -> p q t", t=2 * Wp)
                        in0 = rows[:, :, 1 : 1 + W]
                        in1 = rows[:, :, Wp + 1 : Wp + 1 + W]
                    q0, q1 = r0 // 2, (r0 + rg) // 2
                    q = rg // 2
                    # TensorTensor may read only one input from PSUM: stage
                    # the even rows into SBUF, then max against the PSUM odd
                    # rows. Steady state splits the copy to ACT for engine
                    # parallelism; the last image's short tail chains run
                    # entirely on DVE to avoid cross-engine sem hops.
                    prA = pr_p.tile(
                        [KP, max_rows // 2, W], F32, tag="prA", bufs=4,
                        name=f"prA{img}{si_}{j}",
                    )
                    nc.scalar.copy(prA[:, :q, :], in0)
                    nc.vector.tensor_max(
                        pr_tiles[img][j][:, q0:q1, :], prA[:, :q, :], in1
                    )
                    prs = pr_tiles[img][j][:, q0:q1, :].rearrange("p q w -> p (q w)")
                    pv = pooled_tiles[img][j].rearrange("p (q w) -> p q w", w=WH)[
                        :, q0:q1, :
                    ]
                    nc.vector.tensor_max(pv, prs[:, 0::2], prs[:, 1::2])
                    # stores: whole channel-chunk per image, but per-stretch
                    # for the last image so the tail ships immediately
                    if img == B - 1:
                        nc.vector.tensor_scalar(
                            pv, pv, s2sb[:, j : j + 1], b2sb[:, j : j + 1],
                            Alu.mult, Alu.add,
                        )
                        nc.sync.dma_start(
                            out=dram_ap(
                                y_h,
                                (img * NCHUNK + j) * KP * PO + q0 * WH,
                                [[PO, KP], [1, (q1 - q0) * WH]],
                            ),
                            in_=pooled_tiles[img][j][:, q0 * WH : q1 * WH],
                        )
                    else:
                        nc.vector.tensor_scalar(
                            pv, pv, s2sb[:, j : j + 1], b2sb[:, j : j + 1],
                            Alu.mult, Alu.add,
                        )
                    if img < B - 1 and si_ == len(sts) - 1:
                        nc.sync.dma_start(
                            out=dram_ap(
                                y_h,
                                (img * NCHUNK + j) * KP * PO,
                                [[PO, KP], [1, PO]],
                            ),
                            in_=pooled_tiles[img][j],
                        )

                for j in range(NCHUNK):
                    conv_stretch(hsT_tiles[img], w2v, st, si, j, pool_cb, f"b{img}")
                if si == len(sts) - 1:
                    hsT_tiles.pop(img)

            # --- emission ---
            # image 0: prep each 7-row piece right before the conv1 stretch
            # that needs it. The conv rhs spans both channel planes as one
            # interval hull, so any copy emitted before a stretch becomes a
            # dependency of it — never emit a copy ahead of an earlier
            # stretch.
            for k in range(NP1):
                plo, pn = p0[k]
                prep_span(0, plo, plo + pn, all_dve=True)
                conv1_stretch(0, k)
            for img in range(B):
                if img > 0:
                    for si in range(len(st2)):
                        conv1_stretch(img, si)
                        if img + 1 < B and si < NQ:
                            prep_group(img + 1, si)
                for si in range(len(st2_tail if img == B - 1 else st2)):
                    if img == 0 and B > 1 and si < NQ:
                        prep_group(1, si)
                    conv2_stretch(img, si)

    nc.compile()
    return nc


# ---------------------------------------------------------------------------
# host-side constant prep
# ---------------------------------------------------------------------------


def _prep_consts(w1, beta1, mean1, var1, w2, beta2, mean2, var2):
    import jax
    import jax.numpy as jnp
    from jax import lax
    from concourse import mybir

    fp8np = mybir.dt.np(mybir.dt.float8e4)

    def prep_w(w, j_major=False):
        ws = np.where(np.asarray(w) >= 0, np.float32(1.0), np.float32(-1.0))
        # [3,3,ci,co] -> [p, (j,) tap, ktile, m]; ci = ktile*128+p, co = j*128+m
        wr = ws.reshape(9, 2, KP, NCHUNK, KP)
        wr = wr.transpose((2, 3, 0, 1, 4) if j_major else (2, 0, 3, 1, 4))
        return np.ascontiguousarray(wr).astype(fp8np)

    w1p, w2p = prep_w(w1, j_major=True), prep_w(w2)

    cpu = jax.devices("cpu")[0]
    MAXH = 9 * C
    with jax.default_device(cpu):
        hs = jnp.arange(-MAXH, MAXH + 1, dtype=jnp.float32)
        bn1 = (hs[:, None] - jnp.asarray(mean1)[None, :]) * lax.rsqrt(
            jnp.asarray(var1) + 1e-3
        )[None, :] + jnp.asarray(beta1)[None, :]
        nonneg = np.asarray(bn1 >= 0)
        r2 = np.asarray(lax.rsqrt(jnp.asarray(var2) + 1e-3))

    assert (np.diff(nonneg.astype(np.int8), axis=0) >= 0).all(), "bn1 not monotone"
    kc = np.where(nonneg.any(0), nonneg.argmax(0), 2 * MAXH + 1) - MAXH
    # device psum holds h/2 (x=+-0.5, w=+-1): sign flips at (kc-0.5)/2
    nt1 = (-(kc.astype(np.float64) - 0.5) / 2.0).astype(np.float32)

    s2 = r2.astype(np.float32)
    b2 = (
        np.asarray(beta2, np.float64)
        - np.asarray(mean2, np.float64) * s2.astype(np.float64)
    ).astype(np.float32)

    def to_pj(a):  # [256] -> [128, 2] with c = j*128+p
        return np.ascontiguousarray(a.reshape(NCHUNK, KP).T).astype(np.float32)

    # pack everything into one [128, CONST_B] uint8 image
    cbuf = np.zeros((KP, CONST_B), dtype=np.uint8)

    def put(off, arr):
        by = np.ascontiguousarray(arr).reshape(KP, -1).view(np.uint8)
        cbuf[:, off : off + by.shape[1]] = by

    put(W1J0_OFF, w1p[:, 0])
    put(W1J1_OFF, w1p[:, 1])
    put(W2_OFF, w2p)
    put(NT1_OFF, to_pj(nt1))
    put(S2_OFF, to_pj(s2))
    put(B2_OFF, to_pj(b2))
    return {"cb": cbuf}


# ---------------------------------------------------------------------------
# entry point
# ---------------------------------------------------------------------------

_cached = {}


def _run(inputs, trace=False):
    from concourse import bass_utils

    x = np.asarray(inputs["x"], dtype=np.float32)
    Bt, H, W, _ = x.shape  # 32, 56, 56, 256
    Bc = Bt // N_CORES

    consts = _prep_consts(
        inputs["w1"], inputs["beta1"], inputs["mean1"], inputs["var1"],
        inputs["w2"], inputs["beta2"], inputs["mean2"], inputs["var2"],
    )

    key = (Bc, H, W)
    if key not in _cached:
        _cached[key] = build_program(Bc, H, W)
    nc = _cached[key]

    # channel-major marshaling: [Bt, H*W, C] -> per-core [Bc, C, H*W]
    xcm = np.ascontiguousarray(
        x.reshape(Bt, H * W, C).transpose(0, 2, 1)
    )

    in_maps = []
    for c in range(N_CORES):
        m = dict(consts)
        m["x"] = xcm[c * Bc : (c + 1) * Bc]
        in_maps.append(m)

    res = bass_utils.run_bass_kernel_spmd(
        nc, in_maps, core_ids=list(range(N_CORES)), trace=trace
    )
    PO = (H // 2) * (W // 2)
    # y comes back channel-major [Bc, 2, 128, PO]; restore NHWC
    y = np.concatenate(
        [r["y"].reshape(Bc, C, PO).transpose(0, 2, 1) for r in res.results], axis=0
    )
    y = np.ascontiguousarray(y.reshape(Bt, H // 2, W // 2, C)).astype(np.float32)
    return y, res


def kernel(**inputs):
    y, _ = _run(inputs, trace=False)
    return y
